# revision 27
# baseline (speedup 1.0000x reference)
"""ChunkGNNEncoder Trainium kernel v3: host prep + Bass/Tile kernel builder.

Math (per GCN layer, PyG GCNConv):
  h = x @ W              (dense, per-core nodes)
  g = dinv * h           (dinv = deg^-1/2, deg = in-degree incl self-loop)
  agg[t] = sum_{edges (s,t)} g[s]  +  g[t] (self-loop)  +  deg^1/2 * b
  h' = relu(dinv[t] * agg[t])
Then global mean pool per graph, final linear.

v3 design (vs v2):
  - table rows partition-major (row = p*WSEG + w): the per-half g table
    write is ONE contiguous DMA (per-partition 13KB bursts) instead of 52
    per-window DMAs at 512B bursts; unblocks the AllGather input wait
  - dma_gather consolidated into ~5 calls per (layer, seg) to amortize the
    ~1-2us fixed SWDGE descriptor-gen overhead on GpSimd
  - st chunk masks precomputed on host (bf16, streamed), killing the DVE
    is_equal builds; masks shared between both layers
  - xT pre-layouted on host as [p, w, k, n], streamed in 13-window
    quarters as single big DMAs
  - folds: dinv into x (host); relu(dinv^2*acc) = dinv^2*relu(acc) so L2's
    table g2 = dinv*(h' @ W2) comes out of the W2 matmul directly; dinv
    folded into the pooling matrix; bias init (deg^1/2*b) DMA'd from a
    host table directly into the acc accumulator
"""

import numpy as np
import ml_dtypes
from dataclasses import dataclass, field

import concourse.bass as bass
import concourse.bacc as bacc
import concourse.mybir as mybir
import concourse.tile as tile


@dataclass
class Cfg:
    n_nodes: int = 50000
    n_edges: int = 300000
    n_graphs: int = 64
    in_dim: int = 768
    hid: int = 256
    out_dim: int = 128
    n_cores: int = 8
    nw: int = 52          # windows per core (128 nodes each)
    nseg: int = 2         # source table halves; nw % nseg == 0
    rw: int = 4           # windows per aggregation range (psum granularity)
    sb_ranges: tuple = (5, 4, 4)         # ranges per gather superbatch
    gmax: int = 8         # chunks per dma_gather call (1024-desc ring cap)

    @property
    def p_local(self):
        return self.nw * 128

    @property
    def wseg(self):
        return self.nw // self.nseg

    @property
    def segrows(self):
        return self.wseg * 128

    @property
    def gpc(self):
        return self.n_graphs // self.n_cores

    @property
    def kin(self):
        return self.in_dim // 128

    @property
    def khid(self):
        return self.hid // 128

    @property
    def ranges(self):
        r = []
        w = 0
        while w < self.nw:
            r.append((w, min(w + self.rw, self.nw)))
            w += self.rw
        return r


@dataclass
class Meta:
    C: np.ndarray = None          # [nseg, nw] chunks per (seg, window)
    sbs: list = field(default_factory=list)  # per (q, superbatch) dicts
    tot_chunks: int = 0
    sbmax: int = 0                # max chunks in a superbatch


def host_prep(x, edge_index, batch, W1, b1, W2, b2, Wp, bp, cfg: Cfg):
    N, E, G = cfg.n_nodes, cfg.n_edges, cfg.n_graphs
    NC, NW, NSEG = cfg.n_cores, cfg.nw, cfg.nseg
    WSEG = cfg.wseg
    src = np.asarray(edge_index[0], dtype=np.int64)
    tgt = np.asarray(edge_index[1], dtype=np.int64)
    batch = np.asarray(batch, dtype=np.int64)

    deg = np.bincount(tgt, minlength=N).astype(np.float64) + 1.0
    dinv = (1.0 / np.sqrt(deg)).astype(np.float32)
    dinvinv = np.sqrt(deg).astype(np.float32)

    node_core = batch // cfg.gpc
    core_nodes = [np.nonzero(node_core == c)[0] for c in range(NC)]
    for c in range(NC):
        assert len(core_nodes[c]) <= cfg.p_local

    indeg = np.bincount(tgt, minlength=N) + 1

    # window bin-packing per core: balance in-degree, <=128 nodes/window
    local_row = np.full(N, -1, np.int64)
    node_window = np.full(N, -1, np.int64)   # window only; slots assigned later
    for c in range(NC):
        nodes = core_nodes[c]
        order = np.argsort(-indeg[nodes], kind="stable")
        wload = np.zeros(NW, np.int64)
        wcount = np.zeros(NW, np.int64)
        win_of = np.empty(len(nodes), np.int64)
        for i in order:
            open_w = np.nonzero(wcount < 128)[0]
            w = open_w[np.argmin(wload[open_w])]
            win_of[i] = w
            wcount[w] += 1
            wload[w] += indeg[nodes[i]]
        node_window[nodes] = win_of

    # --- per-source-half in-degree refinement: source half = source's
    # window-half on its home core (w < wseg -> half 0). Swapping two nodes
    # between windows of the SAME half on the same core moves only their
    # in-edge counts (target side); source-half memberships are unchanged,
    # so the refinement has purely local effect. Goal: per (core, half q,
    # window w) in-edge count <= 3*128 so every chunk group bakes C=3.
    src_half = node_window[src] // WSEG              # per-edge source half
    dq = np.zeros((NSEG, N), np.int64)
    for q in range(NSEG):
        np.add.at(dq[q], tgt[src_half == q], 1)
    cap = 3 * 128
    for c in range(NC):
        nodes = core_nodes[c]
        for hw in range(NSEG):                       # window-half being packed
            wlo, whi = hw * WSEG, (hw + 1) * WSEG
            sel = nodes[(node_window[nodes] >= wlo) & (node_window[nodes] < whi)]
            win = node_window[sel] - wlo             # [n] in 0..WSEG
            d = dq[:, sel]                           # [NSEG, n]
            loads = np.zeros((NSEG, WSEG), np.int64)
            for q in range(NSEG):
                np.add.at(loads[q], win, d[q])
            for _ in range(4000):
                worst = np.unravel_index(np.argmax(loads), loads.shape)
                q0, w0 = int(worst[0]), int(worst[1])
                if loads[q0, w0] <= cap - 4:
                    break
                w1 = int(np.argmin(loads[q0]))
                if w1 == w0:
                    break
                cand0 = np.nonzero(win == w0)[0]
                cand1 = np.nonzero(win == w1)[0]
                if len(cand0) == 0 or len(cand1) == 0:
                    break
                bias0 = d[q0, cand0] - d[1 - q0, cand0]
                bias1 = d[q0, cand1] - d[1 - q0, cand1]
                i0 = cand0[np.argmax(bias0)]
                i1 = cand1[np.argmin(bias1)]
                gain = d[q0, i0] - d[q0, i1]
                if gain <= 0:
                    break
                win[i0], win[i1] = w1, w0
                for q in range(NSEG):
                    loads[q, w0] += d[q, i1] - d[q, i0]
                    loads[q, w1] += d[q, i0] - d[q, i1]
            node_window[sel] = win + wlo

    # assign slots within windows
    for c in range(NC):
        nodes = core_nodes[c]
        wcount = np.zeros(NW, np.int64)
        lr = np.empty(len(nodes), np.int64)
        wins = node_window[nodes]
        for i in range(len(nodes)):
            w = wins[i]
            lr[i] = w * 128 + wcount[w]
            wcount[w] += 1
        local_row[nodes] = lr

    # --- real edges only, bucketed by (target core, source half, window)
    # Table rows are partition-major within a (core, seg) block:
    #   trow = p*WSEG + (w - q*WSEG), global idx = core*segrows + trow.
    e_core = node_core[tgt]
    e_lrow_t = local_row[tgt]
    e_w = e_lrow_t // 128
    e_tshift = e_lrow_t % 128
    s_w = local_row[src] // 128
    s_p = local_row[src] % 128
    e_seg = s_w // WSEG                              # source window-half
    e_idx16 = node_core[src] * cfg.segrows + s_p * WSEG + (s_w - e_seg * WSEG)
    assert e_idx16.max() < 32768

    cnt = np.zeros((NC, NSEG, NW), np.int64)
    np.add.at(cnt, (e_core, e_seg, e_w), 1)
    C = np.maximum(0, -(-cnt.max(axis=0) // 128))     # [NSEG, NW]

    meta = Meta()
    meta.C = C

    # chunk order: seg-major -> range -> window -> chunks; superbatches
    # group consecutive ranges (cfg.sb_ranges per seg)
    chunk_order = []
    slot_base = {}
    sbs = []
    rngs = cfg.ranges
    assert sum(cfg.sb_ranges) == len(rngs)
    for q in range(NSEG):
        ri = 0
        for nr in cfg.sb_ranges:
            sb = dict(q=q, base=len(chunk_order), ranges=[])
            for _ in range(nr):
                (w0, w1) = rngs[ri]
                ri += 1
                coff = len(chunk_order) - sb["base"]
                sb["ranges"].append((w0, w1, coff, ri - 1))
                for w in range(w0, w1):
                    for _ in range(C[q, w]):
                        chunk_order.append((q, w))
                    slot_base[(q, w)] = (len(chunk_order) - C[q, w]) * 128
            sb["nchk"] = len(chunk_order) - sb["base"]
            sbs.append(sb)
    meta.sbs = sbs
    meta.tot_chunks = len(chunk_order)
    meta.sbmax = max(sb["nchk"] for sb in sbs)
    TOTC = meta.tot_chunks
    TOTS = TOTC * 128

    dt_bf16 = ml_dtypes.bfloat16
    dt_fp8 = ml_dtypes.float8_e4m3fn
    in_maps = []
    W1b = np.asarray(W1, np.float32).astype(dt_fp8)
    W2b = np.asarray(W2, np.float32).astype(dt_bf16)
    Wpb = np.asarray(Wp, np.float32).astype(dt_bf16)
    b1f = np.asarray(b1, np.float32)
    b2f = np.asarray(b2, np.float32)
    b1_bcast = np.tile(b1f[None, :], (128, 1))        # [128, HID]
    b2_bcast = np.tile(b2f[None, :], (128, 1))
    bpf = np.asarray(bp, np.float32).reshape(1, -1)
    x = np.asarray(x, np.float32)
    xs = x * dinv[:, None]                            # fold dinv into x

    for c in range(NC):
        mask = e_core == c
        cs, cw, ct, cq = (e_seg[mask], e_w[mask], e_tshift[mask],
                          e_idx16[mask])
        idx_flat = np.zeros(TOTS, np.int64)
        tsh_flat = np.full(TOTS, -1, np.int64)
        key = cs * NW + cw
        order = np.argsort(key, kind="stable")
        ks, kt, kq2, kw = cq[order], ct[order], cs[order], cw[order]
        uniq, starts = np.unique(kq2 * NW + kw, return_index=True)
        starts = list(starts) + [len(ks)]
        for u, s0, s1 in zip(uniq, starts[:-1], starts[1:]):
            q, w = int(u) // NW, int(u) % NW
            n = s1 - s0
            b = slot_base[(q, w)]
            assert n <= C[q, w] * 128
            so = np.argsort(ks[s0:s1], kind="stable")
            idx_flat[b:b + n] = ks[s0:s1][so]
            tsh_flat[b:b + n] = kt[s0:s1][so]

        idxp = idx_flat.reshape(-1, 16).T.astype(np.int16)
        idxp = np.tile(idxp, (8, 1))                  # [128, TOTS/16]

        # st masks [slot(part), chunk, tgt-shift] fp8
        tshm = tsh_flat.reshape(TOTC, 128)            # [c, slot]
        st3 = (tshm[:, :, None] == np.arange(128)[None, None, :])
        stp = (st3.transpose(1, 0, 2).reshape(128, TOTC * 128)
               .astype(dt_fp8))

        nodes = core_nodes[c]
        lr = local_row[nodes]
        xT = np.zeros((cfg.in_dim, cfg.p_local), np.float32)
        xT[:, lr] = xs[nodes].T
        # [k, p, w, n] -> [p, w, k, n]
        xprep = (xT.reshape(cfg.kin, 128, NW, 128).transpose(1, 2, 0, 3)
                 .reshape(128, NW * cfg.kin * 128).astype(dt_fp8))

        dinv_l = np.zeros(cfg.p_local, np.float32)
        dinv_l[lr] = dinv[nodes]
        dinv_cols = dinv_l.reshape(NW, 128).T         # [p, w]
        dinvsq_cols = (dinv_cols * dinv_cols).copy().astype(np.float32)
        dinvinv_l = np.zeros(cfg.p_local, np.float32)
        dinvinv_l[lr] = dinvinv[nodes]
        dinvinv_cols = dinvinv_l.reshape(NW, 128).T.copy().astype(np.float32)

        # pooling matrix with dinv folded: pooled = sum dinv*relu(acc2)
        spool = np.zeros((cfg.p_local, cfg.gpc), np.float32)
        gl = batch[nodes] - c * cfg.gpc
        spool[lr, gl] = dinv[nodes]
        spool = (spool.reshape(NW, 128, cfg.gpc).transpose(1, 0, 2)
                 .reshape(128, NW * cfg.gpc)).astype(dt_bf16)
        cnt_g = np.bincount(gl, minlength=cfg.gpc).astype(np.float32)
        cntinv = (1.0 / np.maximum(cnt_g, 1.0)).astype(np.float32)
        cntinv_rep = np.tile(cntinv[None, :], (128, cfg.khid))

        ident = np.eye(128, dtype=dt_bf16)

        in_maps.append(dict(
            xprep=xprep, W1=W1b, W2=W2b, Wp=Wpb,
            b1b=b1_bcast, b2b=b2_bcast,
            bp8=np.tile(bpf, (cfg.gpc, 1)).astype(np.float32),
            dinvsq_cols=dinvsq_cols, dinvinv_cols=dinvinv_cols,
            idx=idxp, st=stp, spool=spool,
            cntinv=cntinv_rep, ident=ident,
        ))
    return in_maps, meta


def build_kernel(cfg: Cfg, meta: Meta, debug=False):
    NC, NW, NSEG = cfg.n_cores, cfg.nw, cfg.nseg
    HID, OUT, GPC = cfg.hid, cfg.out_dim, cfg.gpc
    KIN, KHID = cfg.kin, cfg.khid
    C = meta.C
    TOTC = meta.tot_chunks
    TOTS = TOTC * 128
    SBMAX = meta.sbmax
    bf16, f32 = mybir.dt.bfloat16, mybir.dt.float32
    fp8 = mybir.dt.float8e4
    WSEG = cfg.wseg
    QW = NW // 4                                     # dense quarter windows
    Relu = mybir.ActivationFunctionType.Relu

    nc = bacc.Bacc(None, target_bir_lowering=False, debug=debug,
                   num_devices=NC if NC > 1 else None,
                   num_swdge_queues=4,
                   dynamic_dma_scratch_size=cfg.gmax * 128 * 16)

    dram_in = lambda n, s, d: nc.dram_tensor(n, s, d, kind="ExternalInput")
    xprep_d = dram_in("xprep", [128, NW * KIN * 128], fp8)
    W1_d = dram_in("W1", [cfg.in_dim, HID], fp8)
    W2_d = dram_in("W2", [HID, HID], bf16)
    Wp_d = dram_in("Wp", [HID, OUT], bf16)
    b1b_d = dram_in("b1b", [128, HID], f32)
    b2b_d = dram_in("b2b", [128, HID], f32)
    bp8_d = dram_in("bp8", [GPC, OUT], f32)
    dinvsq_d = dram_in("dinvsq_cols", [128, NW], f32)
    dinvinv_d = dram_in("dinvinv_cols", [128, NW], f32)
    idx_d = dram_in("idx", [128, TOTS // 16], mybir.dt.int16)
    st_d = dram_in("st", [128, TOTC * 128], fp8)
    spool_d = dram_in("spool", [128, NW * GPC], bf16)
    cntinv_d = dram_in("cntinv", [128, KHID * GPC], f32)
    ident_d = dram_in("ident", [128, 128], bf16)
    out_d = nc.dram_tensor("out", [GPC, OUT], f32, kind="ExternalOutput")

    sem_i = [0]

    with tile.TileContext(nc) as tc:
        with (
            tc.tile_pool(name="const", bufs=1) as cpool,
            tc.tile_pool(name="xw", bufs=2) as xwpool,
            tc.tile_pool(name="gbuf", bufs=2) as gpool,
            tc.tile_pool(name="flush", bufs=3) as fpool,
            tc.tile_pool(name="psagg", bufs=2, space="PSUM") as psagg,
            tc.tile_pool(name="psx", bufs=2, space="PSUM") as psx,
            tc.tile_pool(name="pspool", bufs=1, space="PSUM") as pspool,
            tc.tile_pool(name="dram", bufs=1, space="DRAM") as dram,
        ):
            # ---- constants
            W1_t = cpool.tile([128, KIN, HID], fp8)
            nc.sync.dma_start(W1_t[:], W1_d[:].rearrange("(k p) n -> p k n", p=128))
            W2_t = cpool.tile([128, KHID, HID], bf16)
            nc.sync.dma_start(W2_t[:], W2_d[:].rearrange("(k p) n -> p k n", p=128))
            Wp_t = cpool.tile([128, KHID, OUT], bf16)
            nc.sync.dma_start(Wp_t[:], Wp_d[:].rearrange("(k p) n -> p k n", p=128))
            b1b_t = cpool.tile([128, HID], f32)
            nc.sync.dma_start(b1b_t[:], b1b_d[:])
            b2b_t = cpool.tile([128, HID], f32)
            nc.sync.dma_start(b2b_t[:], b2b_d[:])
            bp8_t = cpool.tile([GPC, OUT], f32)
            nc.sync.dma_start(bp8_t[:], bp8_d[:])
            dinvsq_t = cpool.tile([128, NW], f32)
            nc.sync.dma_start(dinvsq_t[:], dinvsq_d[:])
            dinvinv_t = cpool.tile([128, NW], f32)
            nc.sync.dma_start(dinvinv_t[:], dinvinv_d[:])
            idx_t = cpool.tile([128, TOTS // 16], mybir.dt.int16)
            nc.sync.dma_start(idx_t[:], idx_d[:])
            st_t = cpool.tile([128, TOTC * 128], fp8)   # 39.9KB/part resident
            nc.sync.dma_start(st_t[:], st_d[:])
            spool_t = cpool.tile([128, NW * GPC], bf16)
            nc.sync.dma_start(spool_t[:], spool_d[:])
            cntinv_t = cpool.tile([128, KHID * GPC], f32)
            nc.sync.dma_start(cntinv_t[:], cntinv_d[:])
            ident_t = cpool.tile([128, 128], bf16)
            nc.sync.dma_start(ident_t[:], ident_d[:])

            # persistent SBUF state
            acc_t = cpool.tile([128, NW, HID], f32)       # 53KB/part
            g_sb = cpool.tile([128, NW, HID], fp8)        # 13.3KB/part

            def acc_binit(bb_t, w0, w1):
                """acc[:, w0:w1, :] = deg^1/2 (outer) b  on DVE."""
                nc.vector.tensor_tensor(
                    out=acc_t[:, w0:w1, :],
                    in0=bb_t[:].unsqueeze(1).broadcast_to([128, w1 - w0, HID]),
                    in1=dinvinv_t[:, w0:w1].unsqueeze(2)
                        .broadcast_to([128, w1 - w0, HID]),
                    op=mybir.AluOpType.mult)

            # ---- AG tables (DRAM)
            ag_in = [[dram.tile([128, WSEG * HID], fp8, tag=f"agin{l}{q}",
                                name=f"agin{l}{q}")
                      for q in range(NSEG)] for l in range(2)]
            ag_out = [[dram.tile([NC * 128, WSEG * HID], fp8,
                                 tag=f"agout{l}{q}", name=f"agout{l}{q}",
                                 addr_space="Shared" if NC > 1 else "Local")
                       for q in range(NSEG)] for l in range(2)]

            def launch_ag(layer, q):
                if NC == 1:
                    return
                nc.gpsimd.collective_compute(
                    "AllGather", mybir.AluOpType.bypass,
                    replica_groups=[list(range(NC))],
                    ins=[ag_in[layer][q][:].opt()],
                    outs=[ag_out[layer][q][:].opt()],
                )

            def g_write(layer, w0, w1):
                """write windows [w0, w1) of g_sb into their ag_in slices
                (may straddle the seg boundary)."""
                for q in range(NSEG):
                    a = max(w0, q * WSEG)
                    b = min(w1, (q + 1) * WSEG)
                    if a >= b:
                        continue
                    dst = ag_out[layer][q] if NC == 1 else ag_in[layer][q]
                    if NC == 1:
                        dst = dst[0:128, (a - q * WSEG) * HID:
                                  (b - q * WSEG) * HID]
                    else:
                        dst = dst[:, (a - q * WSEG) * HID:
                                  (b - q * WSEG) * HID]
                    nc.sync.dma_start(
                        dst,
                        g_sb[:, a:b, :].rearrange("p w h -> p (w h)"))

            def table_view(layer, q):
                return ag_out[layer][q][:].rearrange(
                    "a (w h) -> (a w) h", h=HID)

            # gather one superbatch (sub-calls capped by the SWDGE ring:
            # dynamic_dma_scratch_size // 16 descriptors)
            GMAX = cfg.gmax

            def issue_sb_gather(layer, sb):
                gb = gpool.tile([128, SBMAX, HID], fp8, tag="gb")
                base, nchk = sb["base"], sb["nchk"]
                ncalls = -(-nchk // GMAX)
                per = -(-nchk // ncalls)
                for g0 in range(0, nchk, per):
                    g1 = min(g0 + per, nchk)
                    nc.gpsimd.dma_gather(
                        gb[:, g0:g1, :], table_view(layer, sb["q"]),
                        idx_t[:, (base + g0) * 8:(base + g1) * 8],
                        num_idxs=(g1 - g0) * 128,
                        num_idxs_reg=(g1 - g0) * 128,
                        elem_size=HID,
                        queue_num=sem_i[0] % 4)
                    sem_i[0] += 1
                return gb

            # =================== L1 dense ================================
            for qt in range(4):
                xq = xwpool.tile([128, QW, KIN, 128], fp8, tag="xq")
                a = qt * QW * KIN * 128
                b = (qt + 1) * QW * KIN * 128
                nc.sync.dma_start(
                    xq[:],
                    xprep_d[:, a:b].rearrange("p (w k n) -> p w k n",
                                              k=KIN, n=128))
                acc_binit(b1b_t, qt * QW, (qt + 1) * QW)
                for wi in range(QW):
                    w = qt * QW + wi
                    psd = psx.tile([128, HID], f32, tag="psx")
                    for k in range(KIN):
                        nc.tensor.matmul(psd[:], xq[:, wi, k, :],
                                         W1_t[:, k, :],
                                         start=(k == 0), stop=(k == KIN - 1))
                    nc.scalar.copy(g_sb[:, w, :], psd[:])
                # self-loop add for the quarter
                nc.vector.tensor_tensor(
                    out=acc_t[:, qt * QW:(qt + 1) * QW, :],
                    in0=acc_t[:, qt * QW:(qt + 1) * QW, :],
                    in1=g_sb[:, qt * QW:(qt + 1) * QW, :],
                    op=mybir.AluOpType.add)
                g_write(0, qt * QW, (qt + 1) * QW)
                if (qt + 1) * QW == WSEG:
                    launch_ag(0, 0)
                if (qt + 1) * QW == NW:
                    launch_ag(0, 1)

            # =================== flush fns ===============================
            def flush_d(w0, w1, ri):
                """L1 acc -> L2 dense -> g2 -> acc2 init, for one range."""
                rw = w1 - w0
                hp = fpool.tile([128, cfg.rw, HID], bf16, tag="hflush")
                for wi in range(rw):
                    w = w0 + wi
                    # hp = relu(dinvsq*acc) = dinvsq*relu(acc)
                    nc.scalar.activation(hp[:, wi, :], acc_t[:, w, :], Relu,
                                         scale=dinvsq_t[:, w:w + 1])
                # bias init for layer 2 (after acc read)
                acc_binit(b2b_t, w0, w1)
                xt2 = fpool.tile([128, cfg.rw, KHID, 128], bf16, tag="xt2")
                for wi in range(rw):
                    for h in range(KHID):
                        pt = psx.tile([128, 128], bf16, tag="psx")
                        nc.tensor.transpose(
                            pt[:], hp[:, wi, h * 128:(h + 1) * 128],
                            ident_t[:])
                        nc.vector.tensor_scalar_mul(xt2[:, wi, h, :],
                                                    pt[:], 1.0)
                for wi in range(rw):
                    w = w0 + wi
                    ps2 = psx.tile([128, HID], f32, tag="psx")
                    for k in range(KHID):
                        nc.tensor.matmul(ps2[:], xt2[:, wi, k, :],
                                         W2_t[:, k, :],
                                         start=(k == 0), stop=(k == KHID - 1))
                    # ps2 already equals g2 (dinvsq folded into hp)
                    nc.scalar.copy(g_sb[:, w, :], ps2[:])
                nc.vector.tensor_tensor(
                    out=acc_t[:, w0:w1, :], in0=acc_t[:, w0:w1, :],
                    in1=g_sb[:, w0:w1, :], op=mybir.AluOpType.add)
                g_write(1, w0, w1)
                if w0 < WSEG <= w1:      # windows 0..WSEG-1 now complete
                    launch_ag(1, 0)
                if w1 == NW:
                    launch_ag(1, 1)

            pooled = pspool.tile([128, KHID * GPC], f32)
            zrow_t = cpool.tile([1, KHID * GPC], bf16)
            nc.vector.memset(zrow_t[:], 0.0)
            # single zeroing init for the whole pooled bank: a later start
            # would wipe the full bank region, clobbering the sibling group
            nc.tensor.matmul(pooled[:], ident_t[0:1, :], zrow_t[:],
                             start=True, stop=False, skip_group_check=True)

            def flush_f(w0, w1, ri):
                """L2 acc -> pool matmuls for one range."""
                rw = w1 - w0
                hp = fpool.tile([128, cfg.rw, HID], bf16, tag="hflush")
                nc.scalar.activation(
                    hp[:, :rw, :].rearrange("p w h -> p (w h)"),
                    acc_t[:, w0:w1, :].rearrange("p w h -> p (w h)"),
                    Relu)
                for wi in range(rw):
                    w = w0 + wi
                    for h in range(KHID):
                        nc.tensor.matmul(
                            pooled[:, h * GPC:(h + 1) * GPC],
                            hp[:, wi, h * 128:(h + 1) * 128],
                            spool_t[:, w * GPC:(w + 1) * GPC],
                            start=False,
                            stop=(w == NW - 1 and h == KHID - 1),
                            skip_group_check=True)

            # =================== agg phases ==============================
            def agg_phase(layer, flush_fn):
                for sb in meta.sbs:
                    gb = issue_sb_gather(layer, sb)
                    base = sb["base"]
                    for (w0, w1, coff, ri) in sb["ranges"]:
                        rw = w1 - w0
                        ps = psagg.tile([128, cfg.rw, HID], f32, tag="psagg")
                        ci = coff
                        for wi in range(rw):
                            w = w0 + wi
                            cq = int(C[sb["q"], w])
                            for k in range(cq):
                                nc.tensor.matmul(
                                    ps[:, wi, :],
                                    st_t[:, (base + ci) * 128:
                                         (base + ci + 1) * 128],
                                    gb[:, ci, :],
                                    start=(k == 0), stop=(k == cq - 1))
                                ci += 1
                        nc.vector.tensor_tensor(
                            out=acc_t[:, w0:w1, :],
                            in0=ps[:, :rw, :],
                            in1=acc_t[:, w0:w1, :],
                            op=mybir.AluOpType.add)
                        if sb["q"] == NSEG - 1 and flush_fn is not None:
                            flush_fn(w0, w1, ri)

            agg_phase(0, flush_d)
            agg_phase(1, flush_f)

            # =================== pooled -> mean -> final =================
            pooledT = fpool.tile([128, KHID * GPC], bf16, tag="pooledT")
            nc.vector.tensor_tensor(out=pooledT[:], in0=pooled[:],
                                    in1=cntinv_t[:], op=mybir.AluOpType.mult)
            ps_out = psx.tile([GPC, OUT], f32, tag="psx")
            for k in range(KHID):
                nc.tensor.matmul(ps_out[:],
                                 pooledT[:, k * GPC:(k + 1) * GPC],
                                 Wp_t[:, k, :],
                                 start=(k == 0), stop=(k == KHID - 1))
            out_sb = fpool.tile([GPC, OUT], f32, tag="outsb")
            nc.vector.tensor_tensor(out=out_sb[:], in0=ps_out[:],
                                    in1=bp8_t[:], op=mybir.AluOpType.add)
            nc.sync.dma_start(out_d[:], out_sb[:])

    nc.compile()
    return nc


def kernel(**inputs) -> "np.ndarray":
    cfg = Cfg()
    in_maps, meta = host_prep(
        inputs["x"], inputs["edge_index"], inputs["batch"],
        inputs["W1"], inputs["b1"], inputs["W2"], inputs["b2"],
        inputs["Wp"], inputs["bp"], cfg)
    nc = build_kernel(cfg, meta, debug=False)
    from concourse.bass_utils import run_bass_kernel_spmd
    res = run_bass_kernel_spmd(nc, in_maps,
                               core_ids=list(range(cfg.n_cores)), trace=False)
    out = np.concatenate([r["out"] for r in res.results], axis=0)
    return np.ascontiguousarray(out.astype(np.float32))
